# revision 1
# baseline (speedup 1.0000x reference)
"""Category-specific 2-layer MLP (MoE-style routing), expert-parallel on 8 NeuronCores.

Math (per sample b with category c = cat_ids[b]):
    h   = relu(x_flat[b] @ W1[c] + b1[c])      x_flat: [32, 4096], W1: [8, 4096, 1024]
    out = h @ W2[c] + b2[c]                    W2: [8, 1024, 512]

Sharding: expert-parallel. Core k holds ONLY category k's weights (16 MB W1 + 2 MB W2)
and computes the full dense MLP for all 32 samples; the host then gathers row b from
core cat_ids[b]. Per-core HBM traffic is ~18.6 MB (the minimum possible when all 8
categories are in use), vs 144 MB for weight replication.

Kernel layout per core (fp32 matmuls stream the MOVING operand at 4 cycles/row, so
keep the moving dim small: stream x^T / h^T at N=32, keep the big weights stationary):
  layer 1: hT[u] [128, 32] (u = 0..7 mid-tiles, one PSUM bank each) accumulated over
           32 K-tiles: lhsT (stationary) = W1[128t:128t+128, 128u:128u+128],
           rhs (moving) = x^T tile [128, 32]. Produces h already transposed for
           layer 2 — no on-chip transpose stage at all.
  bias+relu: ONE DVE scalar_tensor_tensor per mid-tile:
           ht_sb = max(hT_psum + b1T[:, u], 0)   (b1 transposed is per-PARTITION).
  layer 2: transposed too: oT[v] [128, 32] (v = 0..3) over 8 K-tiles:
           lhsT = W2[128u:128u+128, 128v:128v+128], rhs = hT[u] [128, 32];
           evict fuses the b2 add. Output leaves the chip as out^T [512, 32];
           the host gather undoes the transpose for free.
  W1 streams as 7 uneven DMAs ([8,8,8,4,2,1,1] K-tiles) — big slabs amortize
  per-DMA latency, the tiny last slab shortens the post-stream PE tail.

Toolchain constraint: this walrus build allows at most ONE sync-wait command per
instruction. The program is structured so every instruction acquires at most one
new semaphore:
  - every W1/W2 slab lives in its own SBUF tile (no slot reuse -> DMAs carry no waits);
  - the xt DMA is placed 8 positions before the first W1 slab DMA, so both land on the
    same HWDGE queue and one cumulative wait covers both;
  - a leading DVE "touch" of the bias tile acquires its queue semaphore before the
    fused bias ops (which then wait only on PE);
  - the kernel-tail drain is split into single-wait drains (_patch_tail_drain).
Verified by _assert_wait_budget at build time.
"""

import numpy as np

import concourse.bass as bass
import concourse.mybir as mybir
import concourse.tile_sem_assignment as _tsa
from concourse import tile
from concourse.bass_utils import run_bass_kernel_spmd

NUM_CAT = 8
B = 32
IN_DIM = 4096   # 16 * 256
MID = 1024
OUT = 512       # 16 * 32
P = 128
KT1 = IN_DIM // P    # 32 k-tiles for layer 1
KT2 = MID // P       # 8 mid-tiles (layer-1 out / layer-2 contraction)
NT = OUT // P        # 4 out-tiles
SLAB_SIZES = (8, 8, 8, 4, 2, 1, 1)  # k-tiles per W1 DMA; sum == KT1
F32 = mybir.dt.float32

HWDGE_QUEUES = 4


class _PatchHwdgeQueues:
    """Pin Tile's HWDGE round-robin to n queues during scheduling."""

    def __init__(self, n: int):
        self.n = n

    def __enter__(self):
        self._saved = _tsa.NUM_HWDGE_SEMS
        _tsa.NUM_HWDGE_SEMS = self.n
        return self

    def __exit__(self, *exc):
        _tsa.NUM_HWDGE_SEMS = self._saved
        return False


def _patch_tail_drain():
    """Split Tile's kernel-tail drain (one wait per live proc) into a chain of
    single-wait drains: this walrus build caps sync-wait commands per instruction
    and rejects the stock multi-wait drain."""
    if getattr(tile.TileContext, "_tail_drain_patched", False):
        return
    from concourse.vector_clock import ScopedClock, VectorClock

    def _drain_and_barrier(self, tick_clock, wait_clock):
        gc = tick_clock.global_clock
        n = len(gc)
        for p in range(n):
            if gc[p] <= 0:
                continue
            sub = [0] * n
            sub[p] = gc[p]
            d = self.nc.sync.drain()
            wait_clock.add_sem_waits(d.ins, ScopedClock({None: VectorClock(sub)}))
        self.nc.all_engine_barrier()
        assert self.sems is not None
        popped = self.nc._tile_sem_poison_stack.pop()
        assert popped is self._sem_poison
        self.nc.clear_and_free_semaphores(list(self.sems.allocated().values()))
        self.nc.all_engine_barrier()

    tile.TileContext._drain_and_barrier = _drain_and_barrier
    tile.TileContext._tail_drain_patched = True


_patch_tail_drain()


def _build_nc() -> bass.Bass:
    nc = bass.Bass()

    # xt[p, t, b] = x_flat[b, t*128 + p]: K-major layout so each DMA partition line
    # is one contiguous 4 KB segment.
    xt = nc.dram_tensor("xt", [P, KT1, B], F32, kind="ExternalInput")
    w1 = nc.dram_tensor("w1", [IN_DIM, MID], F32, kind="ExternalInput")
    w2 = nc.dram_tensor("w2", [MID, OUT], F32, kind="ExternalInput")
    # biast[p, 0:8] = b1[128u + p]; [p, 8:12] = b2[128v + p]; [p, 12] = 0.
    biast = nc.dram_tensor("biast", [P, KT2 + NT + 1], F32, kind="ExternalInput")
    out = nc.dram_tensor("out", [OUT, B], F32, kind="ExternalOutput")  # transposed

    with _PatchHwdgeQueues(HWDGE_QUEUES), tile.TileContext(nc) as tc:
        with (
            tc.tile_pool(name="const", bufs=1) as const,
            tc.tile_pool(name="w1p", bufs=1) as w1p,
            tc.tile_pool(name="w2p", bufs=1) as w2p,
            tc.tile_pool(name="work", bufs=1) as work,
            tc.tile_pool(name="psum", bufs=1, space="PSUM") as psum,
        ):
            # DMA issue order fixes HWDGE queue assignment (round-robin mod 4):
            # 0:xt 1:biast 2:w2a 3:w2b 4+:w1 slabs. xt (pos 0) and w1 slab 0
            # (pos 4) share a queue -> one cumulative wait covers both for the
            # first matmul. Each queue later carries a W1 slab, so every small
            # input is covered by the slab waits PE already performs.
            xt_sb = const.tile([P, KT1, B], F32)
            nc.sync.dma_start(xt_sb[:], xt[:])
            biast_sb = const.tile([P, KT2 + NT + 1], F32)
            nc.sync.dma_start(biast_sb[:], biast[:])

            # W2 in two 1 MB DMAs; w2_sbs[h][:, f, :] is K-tile 4h+f.
            w2_sbs = []
            for h in range(2):
                w2_sb = w2p.tile([P, KT2 // 2, OUT], F32, tag=f"w2_{h}", name=f"w2sb{h}")
                nc.sync.dma_start(
                    w2_sb[:],
                    w2[P * (KT2 // 2) * h : P * (KT2 // 2) * (h + 1), :].rearrange(
                        "(f p) n -> p f n", p=P
                    ),
                )
                w2_sbs.append(w2_sb)

            # W1 as 8 uneven DMAs; w1_sbs[s][:, f, :] is K-tile (slab_start[s] + f).
            w1_sbs = []
            row = 0
            slab_of_ktile = []
            for s, sz in enumerate(SLAB_SIZES):
                w1_sb = w1p.tile([P, sz, MID], F32, tag=f"w1_{s}", name=f"w1sb{s}")
                nc.sync.dma_start(
                    w1_sb[:],
                    w1[row : row + P * sz, :].rearrange("(f p) n -> p f n", p=P),
                )
                w1_sbs.append(w1_sb)
                slab_of_ktile += [(s, f) for f in range(sz)]
                row += P * sz

            # ---- layer 1: hT[u][128, 32] = (x @ W1)^T mid-tiles, 8 PSUM banks ----
            ht_ps = [
                psum.tile([P, B], F32, tag=f"hT_{u}", name=f"htps{u}")
                for u in range(KT2)
            ]
            for t in range(KT1):
                s, f = slab_of_ktile[t]
                for u in range(KT2):
                    nc.tensor.matmul(
                        ht_ps[u][:],
                        w1_sbs[s][:, f, P * u : P * (u + 1)],
                        xt_sb[:, t, :],
                        start=(t == 0),
                        stop=(t == KT1 - 1),
                    )

            # DVE touch: acquire the biast queue semaphore ahead of the fused
            # bias ops so they only ever wait on PE.
            touch_sb = work.tile([P, 1], F32)
            nc.vector.tensor_copy(touch_sb[:], biast_sb[:, 12:13])

            zero_bc = biast_sb[:, 12:13].to_broadcast((P, B))

            # ---- fused bias+relu evict: ht_sb[:,u,:] = max(hT[u] + b1T[:,u], 0) ----
            ht_sb = work.tile([P, KT2, B], F32)
            for u in range(KT2):
                nc.vector.scalar_tensor_tensor(
                    ht_sb[:, u, :],
                    ht_ps[u][:],
                    biast_sb[:, u : u + 1],
                    zero_bc,
                    mybir.AluOpType.add,
                    mybir.AluOpType.max,
                )

            # ---- layer 2 (transposed): oT[v][128, 32] over 8 K-tiles ----
            # oT psum tiles reuse hT_0..3 slots (released once their evict ran).
            ot_ps = [
                psum.tile([P, B], F32, tag=f"hT_{v}", name=f"otps{v}")
                for v in range(NT)
            ]
            for u in range(KT2):
                for v in range(NT):
                    nc.tensor.matmul(
                        ot_ps[v][:],
                        w2_sbs[u // 4][:, u % 4, P * v : P * (v + 1)],
                        ht_sb[:, u, :],
                        start=(u == 0),
                        stop=(u == KT2 - 1),
                    )

            # ---- fused bias evict: ot_sb[:,v,:] = oT[v] + b2T[:,v] ----
            ot_sb = work.tile([P, NT, B], F32)
            for v in range(NT):
                nc.vector.scalar_tensor_tensor(
                    ot_sb[:, v, :],
                    ot_ps[v][:],
                    biast_sb[:, KT2 + v : KT2 + v + 1],
                    zero_bc,
                    mybir.AluOpType.add,
                    mybir.AluOpType.add,
                )
            # SWDGE path: a fresh DMA proc, so the store carries only the DVE wait
            # (an HWDGE queue would add a self-queue FIFO wait -> 2 waits).
            nc.gpsimd.dma_start(out.rearrange("(v p) b -> p v b", p=P), ot_sb[:])

    _assert_wait_budget(nc)
    return nc


def _assert_wait_budget(nc: bass.Bass, max_waits: int = 1):
    """This walrus build rejects instructions with >1 sync wait; fail fast."""
    bad = []
    for blk in nc.m.functions[0].blocks:
        for inst in blk.instructions:
            if type(inst).__name__ not in (
                "InstMatmult",
                "InstDMACopy",
                "InstDrain",
                "InstTensorCopy",
                "InstTensorScalarPtr",
            ):
                continue
            si = inst.sync_info
            nw = len(si.on_wait) if si is not None else 0
            if nw > max_waits:
                bad.append(
                    (
                        inst.name,
                        type(inst).__name__,
                        [(w.ant_name, w.wait_value) for w in si.on_wait],
                    )
                )
    if bad:
        raise RuntimeError(f"instructions with >{max_waits} sync waits: {bad}")


_NC_CACHE: bass.Bass | None = None


def _get_nc() -> bass.Bass:
    global _NC_CACHE
    if _NC_CACHE is None:
        _NC_CACHE = _build_nc()
    return _NC_CACHE


def _make_in_maps(x, W1, b1, W2, b2):
    x_flat = np.ascontiguousarray(np.asarray(x, dtype=np.float32)).reshape(B, IN_DIM)
    # xt[p, t, b] = x_flat[b, t*128 + p]
    xt = np.ascontiguousarray(x_flat.reshape(B, KT1, P).transpose(2, 1, 0))
    W1 = np.ascontiguousarray(np.asarray(W1, dtype=np.float32))
    W2 = np.ascontiguousarray(np.asarray(W2, dtype=np.float32))
    b1 = np.asarray(b1, dtype=np.float32)
    b2 = np.asarray(b2, dtype=np.float32)
    biast = np.zeros((NUM_CAT, P, KT2 + NT + 1), dtype=np.float32)
    biast[:, :, :KT2] = b1.reshape(NUM_CAT, KT2, P).transpose(0, 2, 1)
    biast[:, :, KT2 : KT2 + NT] = b2.reshape(NUM_CAT, NT, P).transpose(0, 2, 1)
    return [
        {
            "xt": xt,
            "w1": W1[k],
            "w2": W2[k],
            "biast": biast[k],
        }
        for k in range(NUM_CAT)
    ]


def kernel(x, W1, b1, W2, b2, cat_ids) -> np.ndarray:
    nc = _get_nc()
    in_maps = _make_in_maps(x, W1, b1, W2, b2)
    res = run_bass_kernel_spmd(nc, in_maps, list(range(NUM_CAT))).results
    per_cat = np.stack([np.asarray(res[k]["out"]) for k in range(NUM_CAT)])  # [8, OUT, B]
    cat = np.asarray(cat_ids).astype(np.int64).reshape(B)
    sel = per_cat[cat, :, np.arange(B)]  # [B, OUT] (gather undoes the transpose)
    return np.ascontiguousarray(sel.reshape(B, 16, 32).astype(np.float32))



# revision 3
# speedup vs baseline: 3.0704x; 3.0704x over previous
"""Category-specific 2-layer MLP (MoE-style routing), expert-parallel on 8 NeuronCores.

Math (per sample b with category c = cat_ids[b]):
    h   = relu(x_flat[b] @ W1[c] + b1[c])      x_flat: [32, 4096], W1: [8, 4096, 1024]
    out = h @ W2[c] + b2[c]                    W2: [8, 1024, 512]

Sharding: expert-parallel. Core k holds ONLY category k's weights and computes the
full MLP for all 32 samples; the host gathers row b from core cat_ids[b].

The kernel is HBM-bandwidth bound, so the weights are stored compressed:
  - W1 (the 16 MB tensor) is quantized host-side to fp8 E3M4 with a per-output-
    column scale s[col] = 15/max|W1[:,col]| (keeps values in e3m4's normal range
    even if hardware flushes fp8 denormals). 4 MB per core.
  - x, W2, h are fp16 (quantization error ~0.03%). W2: 1 MB per core.
  - Measured end-to-end rel err vs the fp32 reference: ~1.4e-2 (vs 2e-2 gate),
    dominated by the e3m4 quantization noise of W1.
  Per-core HBM traffic: ~5.3 MB vs 18.9 MB for the fp32 version.

Dataflow (transposed layout; matmul cost scales with the MOVING free dim, so the
moving operand is always the narrow [*, 32] activation tile):
  - W1 streams as 8 column slabs (one per 128-wide mid-tile u), each [4096, 128]
    pre-tiled on host to [p=128, t=32, m=128] so DMA lines are 4 KB contiguous.
  - When slab u lands: 32 PE matmuls accumulate hT[u] [128, 32] in a ping-pong
    PSUM bank (lhsT = W1 k-tile fp8, moving = xT fp16 tile).
  - ACT engine evicts: ht_sb[u] = relu(hT[u]*inv_s[u] + b1T[u]) -> fp16, one
    fused activation per mid-tile (scale/bias are per-partition operands).
  - 4 more PE matmuls immediately fold mid-tile u into the layer-2 accumulators
    oT[v] [128, 32] (start suppressed; ACT pre-wrote b2T into the PSUM bank, so
    accumulation starts from the bias). Layer 2 therefore finishes ~0.5 us after
    the last W1 byte arrives; there is no big serial tail.
  - The output leaves PSUM directly via one SWDGE DMA (fp32, [128, 4*32]); the
    host gather undoes the transpose for free.

Toolchain constraint: this walrus build allows at most ONE sync-wait command per
instruction. All input DMAs are issued up front on a single HWDGE queue
(NUM_HWDGE_SEMS patched to 1) in the order biast/w2/xt/w1-slabs, so every
consumer needs only one cumulative wait on that queue's semaphore, and every
other cross-engine dependency is transitively covered by per-engine program
order (PE alternates layer-1/layer-2 per slab; ACT does warm/b2-init/evicts in
order). The kernel-tail drain is split into single-wait drains
(_patch_tail_drain). Verified by _assert_wait_budget at build time.
"""

import ml_dtypes
import numpy as np

import concourse.bass as bass
import concourse.mybir as mybir
import concourse.tile_sem_assignment as _tsa
from concourse import tile
from concourse.bass_utils import run_bass_kernel_spmd

NUM_CAT = 8
B = 32
IN_DIM = 4096   # 16 * 256
MID = 1024
OUT = 512       # 16 * 32
P = 128
KT1 = IN_DIM // P    # 32 k-tiles for layer 1
KT2 = MID // P       # 8 mid-tiles (layer-1 out / layer-2 contraction)
NT = OUT // P        # 4 out-tiles
F32 = mybir.dt.float32
F16 = mybir.dt.float16
F8 = mybir.dt.float8e3  # E3M4
NP_F8 = ml_dtypes.float8_e3m4
FP8_SCALE_TARGET = 15.0  # e3m4 max normal is 15.5


class _PatchHwdgeQueues:
    """Pin Tile's HWDGE round-robin to n queues during scheduling."""

    def __init__(self, n: int):
        self.n = n

    def __enter__(self):
        self._saved = _tsa.NUM_HWDGE_SEMS
        _tsa.NUM_HWDGE_SEMS = self.n
        return self

    def __exit__(self, *exc):
        _tsa.NUM_HWDGE_SEMS = self._saved
        return False


def _patch_tail_drain():
    """Split Tile's kernel-tail drain (one wait per live proc) into a chain of
    single-wait drains: this walrus build caps sync-wait commands per instruction
    and rejects the stock multi-wait drain."""
    if getattr(tile.TileContext, "_tail_drain_patched", False):
        return
    from concourse.vector_clock import ScopedClock, VectorClock

    def _drain_and_barrier(self, tick_clock, wait_clock):
        gc = tick_clock.global_clock
        n = len(gc)
        for p in range(n):
            if gc[p] <= 0:
                continue
            sub = [0] * n
            sub[p] = gc[p]
            d = self.nc.sync.drain()
            wait_clock.add_sem_waits(d.ins, ScopedClock({None: VectorClock(sub)}))
        self.nc.all_engine_barrier()
        assert self.sems is not None
        popped = self.nc._tile_sem_poison_stack.pop()
        assert popped is self._sem_poison
        self.nc.clear_and_free_semaphores(list(self.sems.allocated().values()))
        self.nc.all_engine_barrier()

    tile.TileContext._drain_and_barrier = _drain_and_barrier
    tile.TileContext._tail_drain_patched = True


_patch_tail_drain()


def _build_nc() -> bass.Bass:
    nc = bass.Bass()

    # xt[p, t, b] = x_flat[b, t*128 + p] (fp16): 2 KB contiguous per partition.
    xt = nc.dram_tensor("xt", [P, KT1, B], F16, kind="ExternalInput")
    # w1q[u, p, t, m] = e3m4(W1[128t+p, 128u+m] * s[128u+m]): one contiguous
    # 512 KB column slab per mid-tile u, 4 KB per partition line.
    w1q = nc.dram_tensor("w1q", [KT2, P, KT1, P], F8, kind="ExternalInput")
    # w2t[p, u, n] = W2[128u+p, n] (fp16): k-major, 8 KB per partition.
    w2t = nc.dram_tensor("w2t", [P, KT2, OUT], F16, kind="ExternalInput")
    # biast[p, 0:8] = 1/s[128u+p]; [p, 8:16] = b1[128u+p]; [p, 16:20] = b2[128v+p]
    biast = nc.dram_tensor("biast", [P, 2 * KT2 + NT], F32, kind="ExternalInput")
    # out[p, 32v + b] = (x@W1..@W2 + b2)^T[128v+p, b]
    out = nc.dram_tensor("out", [P, NT * B], F32, kind="ExternalOutput")

    with _PatchHwdgeQueues(1), tile.TileContext(nc) as tc:
        with (
            tc.tile_pool(name="const", bufs=1) as const,
            tc.tile_pool(name="w1p", bufs=1) as w1p,
            tc.tile_pool(name="work", bufs=1) as work,
            tc.tile_pool(name="psum", bufs=1, space="PSUM") as psum,
        ):
            # All input DMAs up front on one HWDGE queue; consumers use a
            # single cumulative wait. Order = transfer schedule (the DMA
            # engines are a serial resource): small/early-needed tensors
            # first, then the W1 slab stream that PE chases.
            biast_sb = const.tile([P, 2 * KT2 + NT], F32)
            nc.sync.dma_start(biast_sb[:], biast[:])                  # pos 1
            w2_sb = const.tile([P, KT2, OUT], F16)
            for h in range(2):
                half = KT2 // 2
                nc.sync.dma_start(                                   # pos 2,3
                    w2_sb[:, h * half : (h + 1) * half, :],
                    w2t[:, h * half : (h + 1) * half, :],
                )
            xt_sb = const.tile([P, KT1, B], F16)
            nc.sync.dma_start(xt_sb[:], xt[:])                       # pos 4
            w1_sbs = []
            for u in range(KT2):
                w1_sb = w1p.tile([P, KT1, P], F8, tag=f"w1_{u}", name=f"w1sb{u}")
                nc.sync.dma_start(w1_sb[:], w1q[u])                  # pos 5+u
                w1_sbs.append(w1_sb)

            # PSUM: 2 ping-pong banks for layer-1 mid-tiles, 1 bank holding all
            # four layer-2 accumulators side by side (fp32 [128, 4*32]).
            ht_ps = [
                psum.tile([P, 512], F32, tag=f"ht_{j}", name=f"htps{j}")
                for j in range(2)
            ]
            ot_ps = psum.tile([P, 512], F32, tag="ot", name="otps")

            # ACT warm-up: load the Relu table off the critical path (first
            # real evict would otherwise eat the ~1.3us table load).
            warm_sb = work.tile([P, 1], F32)
            nc.scalar.activation(
                warm_sb[:],
                biast_sb[:, 0:1],
                mybir.ActivationFunctionType.Relu,
                bias=biast_sb[:, 1:2],
            )
            # ACT pre-writes b2T into the layer-2 accumulators: layer-2
            # matmuls then accumulate on top (start suppressed) and the
            # finished output DMAs straight out of PSUM.
            for v in range(NT):
                nc.scalar.activation(
                    ot_ps[:, v * B : (v + 1) * B],
                    biast_sb[:, 2 * KT2 + v : 2 * KT2 + v + 1].to_broadcast((P, B)),
                    mybir.ActivationFunctionType.Copy,
                )

            ht_sb = work.tile([P, KT2, B], F16)
            for u in range(KT2):
                # layer 1: hT[u][128, 32] accumulated over 32 k-tiles.
                # lhsT (stationary) = fp8 W1 tile, moving = fp16 xT tile.
                hp = ht_ps[u % 2]
                for t in range(KT1):
                    nc.tensor.matmul(
                        hp[:, :B],
                        w1_sbs[u][:, t, :],
                        xt_sb[:, t, :],
                        start=(t == 0),
                        stop=(t == KT1 - 1),
                    )
                # fused dequant+bias+relu evict on the (otherwise idle) ACT
                # engine: ht = relu(hT * inv_s[u] + b1T[u]) -> fp16
                nc.scalar.activation(
                    ht_sb[:, u, :],
                    hp[:, :B],
                    mybir.ActivationFunctionType.Relu,
                    bias=biast_sb[:, KT2 + u : KT2 + u + 1],
                    scale=biast_sb[:, u : u + 1],
                )
                # layer 2: fold mid-tile u into all four oT accumulators.
                for v in range(NT):
                    nc.tensor.matmul(
                        ot_ps[:, v * B : (v + 1) * B],
                        w2_sb[:, u, v * P : (v + 1) * P],
                        ht_sb[:, u, :],
                        start=False,
                        stop=(u == KT2 - 1),
                        skip_group_check=True,
                    )

            # b2 is already folded into PSUM, so the final evict is a single
            # ACT copy of all four accumulators; SWDGE (a fresh DMA proc, so
            # the store carries only the ACT wait) ships it out.
            ot_sb = work.tile([P, NT * B], F32)
            nc.scalar.activation(
                ot_sb[:],
                ot_ps[:, : NT * B],
                mybir.ActivationFunctionType.Copy,
            )
            nc.gpsimd.dma_start(out[:], ot_sb[:])

    _assert_wait_budget(nc)
    return nc


def _assert_wait_budget(nc: bass.Bass, max_waits: int = 1):
    """This walrus build rejects instructions with >1 sync wait; fail fast."""
    bad = []
    for blk in nc.m.functions[0].blocks:
        for inst in blk.instructions:
            if type(inst).__name__ not in (
                "InstMatmult",
                "InstDMACopy",
                "InstDrain",
                "InstTensorCopy",
                "InstTensorScalarPtr",
                "InstActivation",
            ):
                continue
            si = inst.sync_info
            nw = len(si.on_wait) if si is not None else 0
            if nw > max_waits:
                bad.append(
                    (
                        inst.name,
                        type(inst).__name__,
                        [(w.ant_name, w.wait_value) for w in si.on_wait],
                    )
                )
    if bad:
        raise RuntimeError(f"instructions with >{max_waits} sync waits: {bad}")


_NC_CACHE: bass.Bass | None = None


def _get_nc() -> bass.Bass:
    global _NC_CACHE
    if _NC_CACHE is None:
        _NC_CACHE = _build_nc()
    return _NC_CACHE


def _make_in_maps(x, W1, b1, W2, b2):
    x_flat = np.ascontiguousarray(np.asarray(x, dtype=np.float32)).reshape(B, IN_DIM)
    # xt[p, t, b] = x_flat[b, t*128 + p]
    xt = np.ascontiguousarray(
        x_flat.reshape(B, KT1, P).transpose(2, 1, 0).astype(np.float16)
    )
    W1 = np.asarray(W1, dtype=np.float32)
    W2 = np.asarray(W2, dtype=np.float32)
    b1 = np.asarray(b1, dtype=np.float32)
    b2 = np.asarray(b2, dtype=np.float32)

    # Per-output-column fp8 scale: s = 15/max|col| keeps every W1 column in
    # e3m4's normal range; inv_s is folded into the ACT evict.
    colmax = np.abs(W1).max(axis=1)                     # [8, MID]
    s = FP8_SCALE_TARGET / np.where(colmax > 0, colmax, 1.0)
    w1q = (W1 * s[:, None, :]).astype(NP_F8)            # [8, IN_DIM, MID]
    # w1q_t[k, u, p, t, m] = w1q[k, 128t+p, 128u+m]
    w1q_t = np.ascontiguousarray(
        w1q.reshape(NUM_CAT, KT1, P, KT2, P).transpose(0, 3, 2, 1, 4)
    )
    # w2t[k, p, u, n] = W2[k, 128u+p, n]
    w2t = np.ascontiguousarray(
        W2.reshape(NUM_CAT, KT2, P, OUT).transpose(0, 2, 1, 3).astype(np.float16)
    )

    biast = np.zeros((NUM_CAT, P, 2 * KT2 + NT), dtype=np.float32)
    biast[:, :, :KT2] = (1.0 / s).reshape(NUM_CAT, KT2, P).transpose(0, 2, 1)
    biast[:, :, KT2 : 2 * KT2] = b1.reshape(NUM_CAT, KT2, P).transpose(0, 2, 1)
    biast[:, :, 2 * KT2 :] = b2.reshape(NUM_CAT, NT, P).transpose(0, 2, 1)

    return [
        {
            "xt": xt,
            "w1q": w1q_t[k],
            "w2t": w2t[k],
            "biast": biast[k],
        }
        for k in range(NUM_CAT)
    ]


def kernel(x, W1, b1, W2, b2, cat_ids) -> np.ndarray:
    nc = _get_nc()
    in_maps = _make_in_maps(x, W1, b1, W2, b2)
    res = run_bass_kernel_spmd(nc, in_maps, list(range(NUM_CAT))).results
    # res[k]["out"][p, 32v + b] = out_k[b, 128v + p]
    per_cat = np.stack(
        [np.asarray(res[k]["out"]).reshape(P, NT, B) for k in range(NUM_CAT)]
    )  # [8, p, v, b]
    cat = np.asarray(cat_ids).astype(np.int64).reshape(B)
    sel = per_cat[cat, :, :, np.arange(B)]  # [B, p, v]
    full = sel.transpose(0, 2, 1).reshape(B, OUT)  # out[b, 128v+p]
    return np.ascontiguousarray(full.reshape(B, 16, 32).astype(np.float32))


# revision 11
# speedup vs baseline: 3.0945x; 1.0078x over previous
"""Category-specific 2-layer MLP (MoE-style routing), expert-parallel on 8 NeuronCores.

Math (per sample b with category c = cat_ids[b]):
    h   = relu(x_flat[b] @ W1[c] + b1[c])      x_flat: [32, 4096], W1: [8, 4096, 1024]
    out = h @ W2[c] + b2[c]                    W2: [8, 1024, 512]

Sharding: expert-parallel. Core k holds ONLY category k's weights and computes the
full MLP for all 32 samples; the host gathers row b from core cat_ids[b].

The kernel is HBM-bandwidth bound, so the weights are stored compressed:
  - W1 (the 16 MB tensor) is quantized host-side to fp8 E3M4 with a per-output-
    column scale s[col] = 15/max|W1[:,col]| (keeps values in e3m4's normal range
    even if hardware flushes fp8 denormals). 4 MB per core.
  - x, W2, h, and the scale/bias vectors are fp16. W2: 1 MB per core.
  - Measured end-to-end rel err vs the fp32 reference: ~1.4e-2 (vs 2e-2 gate),
    dominated by the e3m4 quantization noise of W1.
  Per-core HBM traffic: ~5.3 MB vs 18.9 MB for the fp32 version.

Dataflow (transposed layout; matmul cost scales with the MOVING free dim, so the
moving operand is always the narrow [*, 32] activation tile):
  - One merged fp16 "static" DMA carries W2 + xT + scales/biases (fewer DMAs =
    fewer per-DMA fixed costs on the serial DMA-engine resource).
  - W1 streams as column slabs (mid-tiles u01, u23, u45, u6, then u7 split
    24+8 k-tiles so almost no layer-1 work remains after the last byte), each
    pre-tiled on host to [p=128, t, m=128] so DMA lines are >=3 KB contiguous.
  - When a slab lands: 32 PE matmuls per mid-tile accumulate hT[u] [128, 32] in
    ping-pong PSUM banks (lhsT = W1 k-tile fp8, moving = xT fp16 tile).
  - ACT engine evicts: ht_sb[u] = relu(hT[u]*inv_s[u] + b1T[u]) -> fp16, one
    fused activation per mid-tile (scale/bias are per-partition operands).
  - 4 more PE matmuls immediately fold mid-tile u into the layer-2 accumulators
    oT[v] [128, 32] (start suppressed; ACT pre-wrote b2T into the PSUM bank, so
    accumulation starts from the bias). Layer 2 therefore finishes ~0.5 us after
    the last W1 byte arrives; there is no big serial tail.
  - One SWDGE DMA ships the finished [128, 128] block out of SBUF (the
    fancier prep/trigger SWDGE paths need GPSIMD ucode libraries this
    runtime cannot load).

Toolchain constraint: this walrus build allows at most ONE sync-wait command per
instruction. All input DMAs are issued up front on a single HWDGE queue
(NUM_HWDGE_SEMS patched to 1) in the order static/w1-slabs, so every consumer
needs only one cumulative wait on that queue's semaphore, and every other
cross-engine dependency is transitively covered by per-engine program order
(PE alternates layer-1/layer-2 per slab; ACT does warm/b2-init/evicts in
order). The kernel-tail drain is split into single-wait drains with the
DMA-store lane drained last (_patch_tail_drain). Verified by
_assert_wait_budget at build time.
"""

import ml_dtypes
import numpy as np

import concourse.bass as bass
import concourse.mybir as mybir
import concourse.tile_sem_assignment as _tsa
from concourse import tile
from concourse.bass_utils import run_bass_kernel_spmd

NUM_CAT = 8
B = 32
IN_DIM = 4096   # 16 * 256
MID = 1024
OUT = 512       # 16 * 32
P = 128
KT1 = IN_DIM // P    # 32 k-tiles for layer 1
KT2 = MID // P       # 8 mid-tiles (layer-1 out / layer-2 contraction)
NT = OUT // P        # 4 out-tiles
F32 = mybir.dt.float32
F16 = mybir.dt.float16
F8 = mybir.dt.float8e3  # E3M4
I32 = mybir.dt.int32
NP_F8 = ml_dtypes.float8_e3m4
FP8_SCALE_TARGET = 15.0  # e3m4 max normal is 15.5

# static fp16 tensor layout (per-partition columns)
SW_W2 = KT2 * OUT            # 4096: w2t[p, u, n]
SW_XT = KT1 * B              # 1024: xt[p, t, b]
SW_VEC = 2 * KT2 + NT        # 20: inv_s[u], b1t[u], b2t[v]
SW = SW_W2 + SW_XT + SW_VEC  # 5140

# W1 slab pieces: (u_start, n_mids, t_start, n_ts). The last mid-tile is split
# 24+8 k-tiles so only ~8 matmuls of layer-1 work remain after the final byte.
W1_PIECES = (
    (0, 2, 0, KT1),
    (2, 2, 0, KT1),
    (4, 2, 0, KT1),
    (6, 1, 0, KT1),
    (7, 1, 0, 24),
    (7, 1, 24, 8),
)


class _PatchHwdgeQueues:
    """Pin Tile's HWDGE round-robin to n queues during scheduling."""

    def __init__(self, n: int):
        self.n = n

    def __enter__(self):
        self._saved = _tsa.NUM_HWDGE_SEMS
        _tsa.NUM_HWDGE_SEMS = self.n
        return self

    def __exit__(self, *exc):
        _tsa.NUM_HWDGE_SEMS = self._saved
        return False


def _patch_tail_drain():
    """Split Tile's kernel-tail drain (one wait per live proc) into a chain of
    single-wait drains (this walrus build caps sync-wait commands per
    instruction and rejects the stock multi-wait drain), draining the SWDGE
    store lane LAST so the other drains overlap the store's completion."""
    if getattr(tile.TileContext, "_tail_drain_patched", False):
        return
    from concourse.vector_clock import ScopedClock, VectorClock
    from concourse.tile_sem_assignment import PROC_NAME_TO_IDX

    sw_procs = {v for k, v in PROC_NAME_TO_IDX.items() if k.startswith("DMASW")}

    def _drain_and_barrier(self, tick_clock, wait_clock):
        gc = tick_clock.global_clock
        n = len(gc)
        order = sorted(range(n), key=lambda p: p in sw_procs)
        for p in order:
            if gc[p] <= 0:
                continue
            sub = [0] * n
            sub[p] = gc[p]
            d = self.nc.sync.drain()
            wait_clock.add_sem_waits(d.ins, ScopedClock({None: VectorClock(sub)}))
        self.nc.all_engine_barrier()
        assert self.sems is not None
        popped = self.nc._tile_sem_poison_stack.pop()
        assert popped is self._sem_poison
        self.nc.clear_and_free_semaphores(list(self.sems.allocated().values()))
        self.nc.all_engine_barrier()

    tile.TileContext._drain_and_barrier = _drain_and_barrier
    tile.TileContext._tail_drain_patched = True


_patch_tail_drain()


def _build_nc() -> bass.Bass:
    nc = bass.Bass()

    # static[p, 0:4096] = W2[128u+p, n] (u-major); [p, 4096:5120] = x_flat
    # transposed (t-major); [p, 5120:5140] = inv_s / b1T / b2T. All fp16.
    static = nc.dram_tensor("static", [P, SW], F16, kind="ExternalInput")
    # w1q[u, p, t, m] = e3m4(W1[128t+p, 128u+m] * s[128u+m]): one contiguous
    # column slab per mid-tile u, 4 KB per partition line.
    w1q = nc.dram_tensor("w1q", [KT2, P, KT1, P], F8, kind="ExternalInput")
    # out[p, 32v + b] = (x@W1..@W2 + b2)^T[128v+p, b]
    out = nc.dram_tensor("out", [P, NT * B], F32, kind="ExternalOutput")

    with _PatchHwdgeQueues(1), tile.TileContext(nc) as tc:
        with (
            tc.tile_pool(name="const", bufs=1) as const,
            tc.tile_pool(name="w1p", bufs=1) as w1p,
            tc.tile_pool(name="work", bufs=1) as work,
            tc.tile_pool(name="psum", bufs=1, space="PSUM") as psum,
        ):
            # All input DMAs up front on one HWDGE queue; consumers use a
            # single cumulative wait. Order = transfer schedule (the DMA
            # engines are a serial resource).
            static_sb = const.tile([P, SW], F16)
            nc.sync.dma_start(static_sb[:], static[:])               # pos 1
            w1_sbs = []
            for i, (u0, nu, t0, nt) in enumerate(W1_PIECES):         # pos 2+i
                w1_sb = w1p.tile([P, nu * nt, P], F8, tag=f"w1_{i}", name=f"w1sb{i}")
                src = w1q[u0 : u0 + nu, :, t0 : t0 + nt, :].rearrange(
                    "u p t m -> p u t m"
                )
                nc.sync.dma_start(
                    w1_sb[:].rearrange("p (u t) m -> p u t m", u=nu), src
                )
                w1_sbs.append(w1_sb)

            w2_v = static_sb[:, :SW_W2].rearrange("p (u n) -> p u n", u=KT2)
            vec0 = SW_W2 + SW_XT
            # The ACT scale/bias operands must be fp32 APs (walrus BIR
            # verifier); the vectors travel as fp16 in the merged static DMA
            # and one idle-DVE copy upconverts them on-chip.
            biast32 = work.tile([P, SW_VEC], F32)
            nc.vector.tensor_copy(biast32[:], static_sb[:, vec0:])
            xt_v = static_sb[:, SW_W2 : SW_W2 + SW_XT].rearrange(
                "p (t b) -> p t b", t=KT1
            )

            # PSUM: 2 ping-pong banks for layer-1 mid-tiles, 1 bank holding all
            # four layer-2 accumulators side by side (fp32 [128, 4*32]).
            ht_ps = [
                psum.tile([P, 512], F32, tag=f"ht_{j}", name=f"htps{j}")
                for j in range(2)
            ]
            ot_ps = psum.tile([P, 512], F32, tag="ot", name="otps")

            # ACT warm-up: load the Relu table off the critical path (first
            # real evict would otherwise eat the ~1.3us table load).
            warm_sb = work.tile([P, 1], F32)
            nc.scalar.activation(
                warm_sb[:],
                biast32[:, 0:1],
                mybir.ActivationFunctionType.Relu,
                bias=biast32[:, 1:2],
            )
            # ACT pre-writes b2T into the layer-2 accumulators: layer-2
            # matmuls then accumulate on top (start suppressed) and the
            # finished output needs no separate bias pass.
            for v in range(NT):
                nc.scalar.activation(
                    ot_ps[:, v * B : (v + 1) * B],
                    biast32[:, 2 * KT2 + v : 2 * KT2 + v + 1].to_broadcast((P, B)),
                    mybir.ActivationFunctionType.Copy,
                )

            ot_sb = work.tile([P, 1, NT * B], F32)

            ht_sb = work.tile([P, KT2, B], F16)

            def l1_matmuls(i):
                u0, nu, t0, nt = W1_PIECES[i]
                for du in range(nu):
                    u = u0 + du
                    hp = ht_ps[u % 2]
                    for dt in range(nt):
                        t = t0 + dt
                        nc.tensor.matmul(
                            hp[:, :B],
                            w1_sbs[i][:, du * nt + dt, :],
                            xt_v[:, t, :],
                            start=(t == 0),
                            stop=(t == KT1 - 1),
                        )
                    if t0 + nt < KT1:
                        continue  # mid-tile not finished (split piece)
                    # fused dequant+bias+relu evict on the ACT engine:
                    # ht = relu(hT * inv_s[u] + b1T[u]) -> fp16
                    nc.scalar.activation(
                        ht_sb[:, u, :],
                        hp[:, :B],
                        mybir.ActivationFunctionType.Relu,
                        bias=biast32[:, KT2 + u : KT2 + u + 1],
                        scale=biast32[:, u : u + 1],
                    )
                    # layer 2: fold mid-tile u into all four oT accumulators.
                    for v in range(NT):
                        nc.tensor.matmul(
                            ot_ps[:, v * B : (v + 1) * B],
                            w2_v[:, u, v * P : (v + 1) * P],
                            ht_sb[:, u, :],
                            start=False,
                            stop=(u == KT2 - 1),
                            skip_group_check=True,
                        )

            for i in range(len(W1_PIECES)):
                l1_matmuls(i)

            # b2 is already folded into PSUM, so the final evict is a single
            # ACT copy of all four accumulators; the pre-generated store
            # descriptors then fire with a cheap trigger.
            nc.scalar.activation(
                ot_sb[:, 0, :],
                ot_ps[:, : NT * B],
                mybir.ActivationFunctionType.Copy,
            )
            # SWDGE store (a fresh DMA proc, so it carries only the ACT wait).
            nc.gpsimd.dma_start(out[:], ot_sb[:, 0, :])

    _assert_wait_budget(nc)
    return nc


def _assert_wait_budget(nc: bass.Bass, max_waits: int = 1):
    """This walrus build rejects instructions with >1 sync wait; fail fast."""
    bad = []
    for blk in nc.m.functions[0].blocks:
        for inst in blk.instructions:
            if type(inst).__name__ not in (
                "InstMatmult",
                "InstDMACopy",
                "InstDrain",
                "InstTensorCopy",
                "InstTensorScalarPtr",
                "InstActivation",
                "InstIota",
                "InstDMAScatterAddAnt",
                "InstTriggerDma",
            ):
                continue
            si = inst.sync_info
            nw = len(si.on_wait) if si is not None else 0
            if nw > max_waits:
                bad.append(
                    (
                        inst.name,
                        type(inst).__name__,
                        [(w.ant_name, w.wait_value) for w in si.on_wait],
                    )
                )
    if bad:
        raise RuntimeError(f"instructions with >{max_waits} sync waits: {bad}")


_NC_CACHE: bass.Bass | None = None


def _get_nc() -> bass.Bass:
    global _NC_CACHE
    if _NC_CACHE is None:
        _NC_CACHE = _build_nc()
    return _NC_CACHE


def _make_in_maps(x, W1, b1, W2, b2):
    x_flat = np.ascontiguousarray(np.asarray(x, dtype=np.float32)).reshape(B, IN_DIM)
    # xt[p, t, b] = x_flat[b, t*128 + p]
    xt = x_flat.reshape(B, KT1, P).transpose(2, 1, 0)  # [p, t, b]
    W1 = np.asarray(W1, dtype=np.float32)
    W2 = np.asarray(W2, dtype=np.float32)
    b1 = np.asarray(b1, dtype=np.float32)
    b2 = np.asarray(b2, dtype=np.float32)

    # Per-output-column fp8 scale: s = 15/max|col| keeps every W1 column in
    # e3m4's normal range; inv_s is folded into the ACT evict.
    colmax = np.abs(W1).max(axis=1)                     # [8, MID]
    s = FP8_SCALE_TARGET / np.where(colmax > 0, colmax, 1.0)
    w1q = (W1 * s[:, None, :]).astype(NP_F8)            # [8, IN_DIM, MID]
    # w1q_t[k, u, p, t, m] = w1q[k, 128t+p, 128u+m]
    w1q_t = np.ascontiguousarray(
        w1q.reshape(NUM_CAT, KT1, P, KT2, P).transpose(0, 3, 2, 1, 4)
    )
    # w2t[k, p, u, n] = W2[k, 128u+p, n]
    w2t = W2.reshape(NUM_CAT, KT2, P, OUT).transpose(0, 2, 1, 3)  # [k, p, u, n]

    static = np.zeros((NUM_CAT, P, SW), dtype=np.float16)
    static[:, :, :SW_W2] = w2t.reshape(NUM_CAT, P, SW_W2)
    static[:, :, SW_W2 : SW_W2 + SW_XT] = (
        xt.reshape(1, P, SW_XT).astype(np.float16)
    )
    vec0 = SW_W2 + SW_XT
    static[:, :, vec0 : vec0 + KT2] = (
        (1.0 / s).reshape(NUM_CAT, KT2, P).transpose(0, 2, 1)
    )
    static[:, :, vec0 + KT2 : vec0 + 2 * KT2] = (
        b1.reshape(NUM_CAT, KT2, P).transpose(0, 2, 1)
    )
    static[:, :, vec0 + 2 * KT2 :] = (
        b2.reshape(NUM_CAT, NT, P).transpose(0, 2, 1)
    )

    return [
        {
            "static": static[k],
            "w1q": w1q_t[k],
        }
        for k in range(NUM_CAT)
    ]


def kernel(x, W1, b1, W2, b2, cat_ids) -> np.ndarray:
    nc = _get_nc()
    in_maps = _make_in_maps(x, W1, b1, W2, b2)
    res = run_bass_kernel_spmd(nc, in_maps, list(range(NUM_CAT))).results
    # res[k]["out"][p, 32v + b] = out_k[b, 128v + p]
    per_cat = np.stack(
        [np.asarray(res[k]["out"]).reshape(P, NT, B) for k in range(NUM_CAT)]
    )  # [8, p, v, b]
    cat = np.asarray(cat_ids).astype(np.int64).reshape(B)
    sel = per_cat[cat, :, :, np.arange(B)]  # [B, p, v]
    full = sel.transpose(0, 2, 1).reshape(B, OUT)  # out[b, 128v+p]
    return np.ascontiguousarray(full.reshape(B, 16, 32).astype(np.float32))
